# revision 26
# baseline (speedup 1.0000x reference)
"""Bahdanau attention kernel for 8 TRN2 NeuronCores.

Problem (per reference):
  B, N, D, H = 128, 2048, 512, 512
  inp  = x @ Wi.T + bi                          [B, H]
  ctx  = context @ Wc.T + bc                    [B, N, H]
  att  = V . tanh(inp + ctx)  (masked)          [B, N]
  alpha = softmax(att); log_p = log_softmax(att)
  hidden = einsum('bnh,bn->bh', ctx, alpha)

Sharding: data-parallel over B (16 batches per core), weights replicated.

Device layout choice: contraction dim D lives on SBUF partitions, so the host
pre-transposes context to [B, D, N].  ctx is computed as ctx^T tiles
[h_chunk(128), n(1024)] in PSUM via float32r matmuls (full-rate fp32).  The
V-dot rides on the PE with one-hot-column weights accumulating a whole
2-batch group's att rows into one [8, 512] PSUM bank.  Softmax runs rowwise
([8,512] = (batch, n_tile) rows) with a cross-partition fixup through tiny
DRAM round-trips.  The alpha-weighted context mix runs on the Vector engine
as fused multiply+row-reduce (scalar_tensor_tensor with accum_out), with
alpha broadcast to 128 partitions via a DRAM bounce.  bc is added to hidden
at the end (sum(alpha) == 1); bias folding otherwise rides the ScalarE
activation ops.
"""

import contextlib

import numpy as np

B, N, D, H = 128, 2048, 512, 512
NCORES = 8
BPC = B // NCORES          # batches per core = 16
NGROUPS = BPC // 2         # softmax groups of 2 batches = 8
NT = N // 512              # n tiles of 512 = 4
DC = D // 128              # d chunks = 4
HC = H // 128              # h chunks = 4

_CACHE = {}


def _build_nc():
    import concourse.bass as bass
    import concourse.bacc as bacc
    import concourse.tile as tile
    from concourse import mybir

    F32 = mybir.dt.float32
    F32R = mybir.dt.float32r
    BF16 = mybir.dt.bfloat16
    AF = mybir.ActivationFunctionType
    ALU = mybir.AluOpType
    AX = mybir.AxisListType

    nc = bacc.Bacc("TRN2", target_bir_lowering=False, debug=False,
                   num_devices=NCORES)

    ctxt_e = nc.declare_dram_parameter("ctxt", [BPC, D, N], BF16, isOutput=False)
    wct_e = nc.declare_dram_parameter("wct", [128, DC, H], BF16, isOutput=False)
    wit_e = nc.declare_dram_parameter("wit", [128, DC, H], BF16, isOutput=False)
    xt_e = nc.declare_dram_parameter("xt", [128, DC, BPC], BF16, isOutput=False)
    bi_e = nc.declare_dram_parameter("bi_p", [128, HC], F32, isOutput=False)
    bc_e = nc.declare_dram_parameter("bc_p", [128, HC], F32, isOutput=False)
    v_e = nc.declare_dram_parameter("v_p", [128, HC], F32, isOutput=False)
    mb_e = nc.declare_dram_parameter("mb", [8, NGROUPS, 512], F32, isOutput=False)
    sel_e = nc.declare_dram_parameter("sel", [8, 8, 128], BF16, isOutput=False)

    hid_e = nc.declare_dram_parameter("hidden", [BPC, H], F32, isOutput=True)
    alp_e = nc.declare_dram_parameter("alpha", [BPC, N], F32, isOutput=True)
    lgp_e = nc.declare_dram_parameter("logp", [BPC, N], F32, isOutput=True)

    # DRAM bounce buffers for cross-partition data movement
    scr_stats = nc.dram_tensor("scr_stats", [NGROUPS, 8, 2], F32)
    scr_scale = nc.dram_tensor("scr_scale", [NGROUPS, 8], F32)
    scr_b2g = nc.dram_tensor("scr_b2g", [NGROUPS, 8], F32)

    def bcast_ap(dram_ap, part_count):
        """Read a DRAM AP replicated onto `part_count` partitions."""
        return bass.AP(tensor=dram_ap.tensor, offset=dram_ap.offset,
                       ap=[[0, part_count]] + [list(a) for a in dram_ap.ap])

    with tile.TileContext(nc) as tc, contextlib.ExitStack() as ctx:
        const = ctx.enter_context(tc.tile_pool(name="const", bufs=1))
        stream = ctx.enter_context(tc.tile_pool(name="stream", bufs=12))
        keep = ctx.enter_context(tc.tile_pool(name="keep", bufs=4))
        tpool = ctx.enter_context(tc.tile_pool(name="tpool", bufs=3))
        mid = ctx.enter_context(tc.tile_pool(name="mid", bufs=2))
        prodp = ctx.enter_context(tc.tile_pool(name="prodp", bufs=1))
        small = ctx.enter_context(tc.tile_pool(name="small", bufs=24))
        ctx_ps = ctx.enter_context(tc.tile_pool(name="ctx_ps", bufs=2, space="PSUM"))
        att_ps = ctx.enter_context(tc.tile_pool(name="att_ps", bufs=2, space="PSUM"))
        arep_ps = ctx.enter_context(tc.tile_pool(name="arep_ps", bufs=2, space="PSUM"))

        # ---- constants ----
        wct = const.tile([128, DC, H], BF16)
        nc.sync.dma_start(out=wct, in_=wct_e.ap())
        wit_a = stream.tile([128, 1024], BF16, tag="ctxt")
        nc.sync.dma_start(out=wit_a, in_=wit_e.ap().rearrange("p c h -> p (c h)")[:, :1024])
        wit_b = stream.tile([128, 1024], BF16, tag="ctxt")
        nc.sync.dma_start(out=wit_b, in_=wit_e.ap().rearrange("p c h -> p (c h)")[:, 1024:])
        wit = None
        xt = const.tile([128, DC, BPC], BF16)
        nc.sync.dma_start(out=xt, in_=xt_e.ap())
        bi_p = const.tile([128, HC], F32)
        nc.sync.dma_start(out=bi_p, in_=bi_e.ap())
        bc_p = const.tile([128, HC], F32)
        nc.sync.dma_start(out=bc_p, in_=bc_e.ap())
        v_p = const.tile([128, HC], F32)
        nc.sync.dma_start(out=v_p, in_=v_e.ap())

        # V one-hot weights: voh[:, c, slot, j] = V_chunk_c if j == slot else 0
        voh = const.tile([128, HC, 8, 8], BF16)
        nc.vector.memset(voh, 0.0)
        for c in range(HC):
            for s in range(8):
                nc.vector.tensor_copy(out=voh[:, c, s, s:s + 1],
                                      in_=v_p[:, c:c + 1])

        # retained across the whole kernel
        attm_all = const.tile([8, NGROUPS, 512], F32)   # masked att rows
        inp_sb = const.tile([128, HC, BPC], F32)        # inp^T chunks
        zeros24 = const.tile([2, 4], F32)
        nc.vector.memset(zeros24, 0.0)
        stats_all = const.tile([2, NGROUPS, 2], F32)
        sel = const.tile([8, 8, 128], BF16)
        nc.sync.dma_start(out=sel, in_=sel_e.ap())

        # ---- phase 1: inp = x @ Wi.T + bi, in [h, b] layout ----
        for c in range(HC):
            ps = ctx_ps.tile([128, 1024], F32, tag="ctxps")
            for d in range(DC):
                wsrc = wit_a if d < 2 else wit_b
                nc.tensor.matmul(ps[:, :BPC],
                                 wsrc[:, (d % 2) * 512 + c * 128:
                                      (d % 2) * 512 + (c + 1) * 128],
                                 xt[:, d, :], start=(d == 0), stop=(d == DC - 1),
                                 skip_group_check=True)
            nc.scalar.activation(out=inp_sb[:, c, :], in_=ps[:, :BPC],
                                 func=AF.Identity, bias=bi_p[:, c:c + 1], scale=1.0)


        def emit_hidden(cks, alpb, g):
            for b2 in range(2):
                b = 2 * g + b2
                ck = cks[b2]
                arep = mid.tile([128, N], BF16, tag="arep")
                for m in range(4):
                    aps = arep_ps.tile([128, 512], F32, tag="areps")
                    nc.tensor.matmul(aps, sel[:, b2 * 4 + m, :], alpb,
                                     start=True, stop=True,
                                     skip_group_check=True)
                    nc.vector.tensor_copy(
                        out=arep[:, m * 512:(m + 1) * 512], in_=aps)
                hacc = small.tile([128, HC], F32, tag="hacc")
                nc.vector.memset(hacc, 0.0)
                prod = prodp.tile([128, N], BF16, tag="prod")
                for c in range(HC):
                    nc.vector.scalar_tensor_tensor(
                        out=prod, in0=ck[:, c, :], scalar=1.0,
                        in1=arep,
                        op0=ALU.mult, op1=ALU.mult,
                        accum_out=hacc[:, c:c + 1])
                nc.vector.tensor_tensor(out=hacc, in0=hacc, in1=bc_p,
                                        op=ALU.add)
                for c in range(HC):
                    nc.sync.dma_start(
                        out=hid_e.ap()[b, c * 128:(c + 1) * 128],
                        in_=hacc[:, c:c + 1])

        pending = None

        # ---- phase 2: main loop ----
        for g in range(NGROUPS):
            attp = att_ps.tile([8, 512], F32, tag="attp")
            cks = []
            for b2 in range(2):
                b = 2 * g + b2
                ctxts = {}
                for npair in range(2):
                    for d in range(DC):
                        ct = stream.tile([128, 1024], BF16, tag="ctxt")
                        nc.sync.dma_start(
                            out=ct,
                            in_=ctxt_e.ap()[b, d * 128:(d + 1) * 128,
                                            npair * 1024:(npair + 1) * 1024])
                        ctxts[(npair, d)] = ct
                ck = keep.tile([128, HC, N], BF16, tag="ck")
                cks.append(ck)
                for npair in range(2):
                    for c in range(HC):
                        ps = ctx_ps.tile([128, 1024], F32, tag="ctxps")
                        for nh in range(2):
                            n0 = npair * 1024 + nh * 512
                            for d in range(DC):
                                nc.tensor.matmul(
                                    ps[:, nh * 512:(nh + 1) * 512],
                                    wct[:, d, c * 128:(c + 1) * 128],
                                    ctxts[(npair, d)][:, nh * 512:(nh + 1) * 512],
                                    start=(d == 0), stop=(d == DC - 1),
                                    skip_group_check=True)
                        t_t = tpool.tile([128, 1024], BF16, tag="t")
                        nc.scalar.activation(out=t_t, in_=ps, func=AF.Tanh,
                                             bias=inp_sb[:, c, b:b + 1], scale=1.0)
                        nc.scalar.activation(
                            out=ck[:, c, npair * 1024:(npair + 1) * 1024],
                            in_=ps, func=AF.Copy)
                        for nh in range(2):
                            m = npair * 2 + nh
                            slot = b2 * 4 + m
                            nc.tensor.matmul(
                                attp, voh[:, c, slot, :],
                                t_t[:, nh * 512:(nh + 1) * 512],
                                start=(b2 == 0 and npair == 0 and c == 0 and nh == 0),
                                stop=(b2 == 1 and npair == 1 and c == HC - 1 and nh == 1),
                                skip_group_check=True)
                if b2 == 0 and pending is not None:
                    emit_hidden(pending[0], pending[1], g - 1)
                    pending = None

            # ---- group softmax: rows r = b2*4 + n_tile ----
            mbg = mid.tile([8, 512], F32, tag="mbg")
            nc.sync.dma_start(out=mbg, in_=mb_e.ap()[:, g, :])
            attm = attm_all[:, g, :]
            nc.vector.tensor_tensor(out=attm, in0=attp, in1=mbg,
                                    op=ALU.add)
            nm8 = small.tile([8, 1], F32, tag="nm8")
            nc.vector.tensor_reduce(out=nm8, in_=attm, axis=AX.X, op=ALU.max,
                                    negate=True)
            e_t = mid.tile([8, 512], F32, tag="e")
            s8 = small.tile([8, 1], F32, tag="s8")
            nc.scalar.activation(out=e_t, in_=attm, func=AF.Exp, bias=nm8,
                                 scale=1.0, accum_out=s8)
            st8 = small.tile([8, 2], F32, tag="st8")
            nc.vector.tensor_copy(out=st8[:, 0:1], in_=nm8)
            nc.vector.tensor_copy(out=st8[:, 1:2], in_=s8)
            nc.sync.dma_start(out=scr_stats[g], in_=st8)
            # per-batch fixup on 2 partitions (b on partitions, n_tile on free)
            stf2 = small.tile([2, 4, 2], F32, tag="stf2")
            nc.sync.dma_start(out=stf2, in_=scr_stats[g])
            nmf2 = stf2[:, :, 0]
            sf2 = stf2[:, :, 1]
            nmb2 = stats_all[:, g, 0:1]
            nc.vector.tensor_reduce(out=nmb2, in_=nmf2, axis=AX.X, op=ALU.min)
            earg = small.tile([2, 4], F32, tag="earg")
            nc.vector.tensor_scalar_sub(out=earg, in0=nmf2, scalar1=nmb2)
            w2 = small.tile([2, 4], F32, tag="w2")
            nc.scalar.activation(out=w2, in_=earg, func=AF.Exp, scale=-1.0)
            sw2 = small.tile([2, 4], F32, tag="sw2")
            nc.vector.tensor_mul(out=sw2, in0=w2, in1=sf2)
            sb2 = stats_all[:, g, 1:2]
            nc.vector.tensor_reduce(out=sb2, in_=sw2, axis=AX.X, op=ALU.add)
            rb2 = small.tile([2, 1], F32, tag="rb2")
            nc.vector.reciprocal(out=rb2, in_=sb2)
            scale2 = small.tile([2, 4], F32, tag="scale2")
            nc.vector.tensor_scalar_mul(out=scale2, in0=w2, scalar1=rb2)
            nc.sync.dma_start(out=scr_scale[g], in_=scale2)
            sclp = small.tile([8, 1], F32, tag="sclp")
            nc.sync.dma_start(out=sclp, in_=scr_scale[g])
            alp = mid.tile([8, 512], F32, tag="alp")
            nc.vector.tensor_scalar_mul(out=alp, in0=e_t, scalar1=sclp)
            alpb = mid.tile([8, 512], BF16, tag="alpb")
            nc.vector.tensor_copy(out=alpb, in_=alp)
            for b2 in range(2):
                b = 2 * g + b2
                nc.sync.dma_start(
                    out=alp_e.ap()[b].rearrange("(m j) -> m j", m=NT),
                    in_=alp[b2 * 4:(b2 + 1) * 4, :])

            pending = (cks, alpb)

        # ---- tail: log_p = attm + (-M_b - ln s_b), one Ln for all groups ----
        lnall = const.tile([2, NGROUPS], F32)
        _ = None
        nc.scalar.activation(out=lnall, in_=stats_all[:, :, 1], func=AF.Ln)
        bias2all = const.tile([2, NGROUPS], F32)
        nc.vector.tensor_sub(out=bias2all, in0=stats_all[:, :, 0], in1=lnall)
        for g in range(NGROUPS):
            b2f = small.tile([2, 4], F32, tag="b2f")
            nc.vector.tensor_scalar_add(out=b2f, in0=zeros24,
                                        scalar1=bias2all[:, g:g + 1])
            nc.sync.dma_start(out=scr_b2g[g], in_=b2f)
            b2rep = small.tile([8, 1], F32, tag="b2rep")
            nc.sync.dma_start(out=b2rep, in_=scr_b2g[g])
            lgp = mid.tile([8, 512], F32, tag="lgp")
            nc.vector.tensor_scalar_add(out=lgp, in0=attm_all[:, g, :],
                                        scalar1=b2rep)
            for b2 in range(2):
                b = 2 * g + b2
                nc.sync.dma_start(
                    out=lgp_e.ap()[b].rearrange("(m j) -> m j", m=NT),
                    in_=lgp[b2 * 4:(b2 + 1) * 4, :])

        if pending is not None:
            emit_hidden(pending[0], pending[1], NGROUPS - 1)
            pending = None

    nc.compile()
    return nc


def _get_nc():
    if "nc" not in _CACHE:
        _CACHE["nc"] = _build_nc()
    return _CACHE["nc"]


def _prep_inputs(x, context, mask, Wi, bi, Wc, bc, V):
    """Host-side sharding + layout prep. Returns per-core input maps."""
    x = np.asarray(x, dtype=np.float32)
    context = np.asarray(context, dtype=np.float32)
    mask_b = np.asarray(mask).reshape(B, N)
    Wi = np.asarray(Wi, dtype=np.float32)
    bi = np.asarray(bi, dtype=np.float32)
    Wc = np.asarray(Wc, dtype=np.float32)
    bc = np.asarray(bc, dtype=np.float32)
    V = np.asarray(V, dtype=np.float32)

    # shared weights
    import ml_dtypes
    wct = np.ascontiguousarray(
        Wc.T.reshape(DC, 128, H).transpose(1, 0, 2)).astype(ml_dtypes.bfloat16)
    wit = np.ascontiguousarray(
        Wi.T.reshape(DC, 128, H).transpose(1, 0, 2)).astype(ml_dtypes.bfloat16)
    bi_p = np.ascontiguousarray(bi.reshape(HC, 128).T)         # [128, HC]
    bc_p = np.ascontiguousarray(bc.reshape(HC, 128).T)
    v_p = np.ascontiguousarray(V.reshape(HC, 128).T)

    mbias = np.where(mask_b, np.float32(-1e30), np.float32(0.0)).astype(np.float32)

    sel = np.zeros((8, 8, 128), dtype=ml_dtypes.bfloat16)
    for r in range(8):
        sel[r, r, :] = 1.0

    in_maps = []
    for i in range(NCORES):
        sl = slice(i * BPC, (i + 1) * BPC)
        ctxt = np.ascontiguousarray(
            context[sl].transpose(0, 2, 1)).astype(ml_dtypes.bfloat16)
        xt = np.ascontiguousarray(
            x[sl].T.reshape(DC, 128, BPC).transpose(1, 0, 2)).astype(ml_dtypes.bfloat16)
        mb_core = mbias[sl].reshape(NGROUPS, 2, NT, 512)
        mb_core = np.ascontiguousarray(
            mb_core.transpose(1, 2, 0, 3).reshape(8, NGROUPS, 512))
        in_maps.append({
            "ctxt": ctxt, "wct": wct, "wit": wit, "xt": xt,
            "bi_p": bi_p, "bc_p": bc_p, "v_p": v_p, "mb": mb_core,
            "sel": sel,
        })
    return in_maps, mask_b


def run(x, context, mask, Wi, bi, Wc, bc, V, trace=False, tmpdir=None):
    from concourse.bass_utils import run_bass_kernel_spmd

    nc = _get_nc()
    in_maps, mask_b = _prep_inputs(x, context, mask, Wi, bi, Wc, bc, V)
    kw = {}
    if trace:
        kw = dict(trace=True, tmpdir=tmpdir)
    res = run_bass_kernel_spmd(nc, in_maps, list(range(NCORES)), **kw)

    hidden = np.empty((B, H), dtype=np.float32)
    alpha = np.empty((B, N), dtype=np.float32)
    log_p = np.empty((B, N), dtype=np.float32)
    for i in range(NCORES):
        sl = slice(i * BPC, (i + 1) * BPC)
        r = res.results[i]
        hidden[sl] = r["hidden"]
        alpha[sl] = r["alpha"]
        log_p[sl] = r["logp"]
    alpha = np.where(mask_b, np.float32(0.0), alpha)
    log_p = np.where(mask_b, np.float32(-np.inf), log_p)
    return (hidden, alpha, log_p), res


def kernel(x, context, mask, Wi, bi, Wc, bc, V):
    out, _ = run(x, context, mask, Wi, bi, Wc, bc, V, trace=False)
    return out


# revision 29
# speedup vs baseline: 1.1158x; 1.1158x over previous
"""Bahdanau attention kernel for 8 TRN2 NeuronCores.

Problem (per reference):
  B, N, D, H = 128, 2048, 512, 512
  inp  = x @ Wi.T + bi                          [B, H]
  ctx  = context @ Wc.T + bc                    [B, N, H]
  att  = V . tanh(inp + ctx)  (masked)          [B, N]
  alpha = softmax(att); log_p = log_softmax(att)
  hidden = einsum('bnh,bn->bh', ctx, alpha)

Sharding: data-parallel over B (16 batches per core), weights replicated.

Device layout choice: contraction dim D lives on SBUF partitions, so the host
pre-transposes context to [B, D, N].  ctx is computed as ctx^T tiles
[h_chunk(128), n(1024)] in PSUM via float32r matmuls (full-rate fp32).  The
V-dot rides on the PE with one-hot-column weights accumulating a whole
2-batch group's att rows into one [8, 512] PSUM bank.  Softmax runs rowwise
([8,512] = (batch, n_tile) rows) with a cross-partition fixup through tiny
DRAM round-trips.  The alpha-weighted context mix runs on the Vector engine
as fused multiply+row-reduce (scalar_tensor_tensor with accum_out), with
alpha broadcast to 128 partitions via a DRAM bounce.  bc is added to hidden
at the end (sum(alpha) == 1); bias folding otherwise rides the ScalarE
activation ops.
"""

import contextlib

import numpy as np

B, N, D, H = 128, 2048, 512, 512
NCORES = 8
BPC = B // NCORES          # batches per core = 16
NGROUPS = BPC // 2         # softmax groups of 2 batches = 8
NT = N // 512              # n tiles of 512 = 4
DC = D // 128              # d chunks = 4
HC = H // 128              # h chunks = 4

_CACHE = {}


def _build_nc():
    import concourse.bass as bass
    import concourse.bacc as bacc
    import concourse.tile as tile
    from concourse import mybir

    F32 = mybir.dt.float32
    F32R = mybir.dt.float32r
    BF16 = mybir.dt.bfloat16
    AF = mybir.ActivationFunctionType
    ALU = mybir.AluOpType
    AX = mybir.AxisListType

    nc = bacc.Bacc("TRN2", target_bir_lowering=False, debug=False,
                   num_devices=NCORES)

    ctxt_e = nc.declare_dram_parameter("ctxt", [BPC, D, N], BF16, isOutput=False)
    wct_e = nc.declare_dram_parameter("wct", [128, DC, H], BF16, isOutput=False)
    wit_e = nc.declare_dram_parameter("wit", [128, DC, H], BF16, isOutput=False)
    xt_e = nc.declare_dram_parameter("xt", [128, DC, BPC], BF16, isOutput=False)
    bi_e = nc.declare_dram_parameter("bi_p", [128, HC], F32, isOutput=False)
    bc_e = nc.declare_dram_parameter("bc_p", [128, HC], F32, isOutput=False)
    v_e = nc.declare_dram_parameter("v_p", [128, HC], F32, isOutput=False)
    mb_e = nc.declare_dram_parameter("mb", [8, NGROUPS, 512], F32, isOutput=False)
    sel_e = nc.declare_dram_parameter("sel", [8, 8, 128], BF16, isOutput=False)

    hid_e = nc.declare_dram_parameter("hidden", [BPC, H], F32, isOutput=True)
    alp_e = nc.declare_dram_parameter("alpha", [BPC, N], F32, isOutput=True)
    lgp_e = nc.declare_dram_parameter("logp", [BPC, N], F32, isOutput=True)

    # DRAM bounce buffers for cross-partition data movement
    scr_stats = nc.dram_tensor("scr_stats", [NGROUPS, 8, 2], F32)
    scr_scale = nc.dram_tensor("scr_scale", [NGROUPS, 8], F32)
    scr_b2g = nc.dram_tensor("scr_b2g", [NGROUPS, 8], F32)

    def bcast_ap(dram_ap, part_count):
        """Read a DRAM AP replicated onto `part_count` partitions."""
        return bass.AP(tensor=dram_ap.tensor, offset=dram_ap.offset,
                       ap=[[0, part_count]] + [list(a) for a in dram_ap.ap])

    with tile.TileContext(nc) as tc, contextlib.ExitStack() as ctx:
        const = ctx.enter_context(tc.tile_pool(name="const", bufs=1))
        stream = ctx.enter_context(tc.tile_pool(name="stream", bufs=6))
        keep = ctx.enter_context(tc.tile_pool(name="keep", bufs=4))
        tpool = ctx.enter_context(tc.tile_pool(name="tpool", bufs=3))
        mid = ctx.enter_context(tc.tile_pool(name="mid", bufs=2))
        prodp = ctx.enter_context(tc.tile_pool(name="prodp", bufs=1))
        small = ctx.enter_context(tc.tile_pool(name="small", bufs=24))
        ctx_ps = ctx.enter_context(tc.tile_pool(name="ctx_ps", bufs=2, space="PSUM"))
        att_ps = ctx.enter_context(tc.tile_pool(name="att_ps", bufs=2, space="PSUM"))
        arep_ps = ctx.enter_context(tc.tile_pool(name="arep_ps", bufs=2, space="PSUM"))

        # ---- constants ----
        wct = const.tile([128, DC, H], BF16)
        nc.sync.dma_start(out=wct, in_=wct_e.ap())
        wit = stream.tile([128, DC, H], BF16, tag="ctxt")
        nc.sync.dma_start(out=wit, in_=wit_e.ap())
        xt = const.tile([128, DC, BPC], BF16)
        nc.sync.dma_start(out=xt, in_=xt_e.ap())
        bi_p = const.tile([128, HC], F32)
        nc.sync.dma_start(out=bi_p, in_=bi_e.ap())
        bc_p = const.tile([128, HC], F32)
        nc.sync.dma_start(out=bc_p, in_=bc_e.ap())
        v_p = const.tile([128, HC], F32)
        nc.sync.dma_start(out=v_p, in_=v_e.ap())

        # V one-hot weights: voh[:, c, slot, j] = V_chunk_c if j == slot else 0
        voh = const.tile([128, HC, 8, 8], BF16)
        nc.vector.memset(voh, 0.0)
        for c in range(HC):
            for s in range(8):
                nc.vector.tensor_copy(out=voh[:, c, s, s:s + 1],
                                      in_=v_p[:, c:c + 1])

        # retained across the whole kernel
        attm_all = const.tile([8, NGROUPS, 512], F32)   # masked att rows
        inp_sb = const.tile([128, HC, BPC], F32)        # inp^T chunks
        zeros24 = const.tile([2, 4], F32)
        nc.vector.memset(zeros24, 0.0)
        stats_all = const.tile([2, NGROUPS, 2], F32)
        sel = const.tile([8, 8, 128], BF16)
        nc.sync.dma_start(out=sel, in_=sel_e.ap())

        # ---- phase 1: inp = x @ Wi.T + bi, in [h, b] layout ----
        for c in range(HC):
            ps = ctx_ps.tile([128, 1024], F32, tag="ctxps")
            for d in range(DC):
                nc.tensor.matmul(ps[:, :BPC], wit[:, d, c * 128:(c + 1) * 128],
                                 xt[:, d, :], start=(d == 0), stop=(d == DC - 1),
                                 skip_group_check=True)
            nc.scalar.activation(out=inp_sb[:, c, :], in_=ps[:, :BPC],
                                 func=AF.Identity, bias=bi_p[:, c:c + 1], scale=1.0)


        def emit_hidden(cks, alpb, g):
            for b2 in range(2):
                b = 2 * g + b2
                ck = cks[b2]
                arep = mid.tile([128, N], BF16, tag="arep")
                for m in range(4):
                    aps = arep_ps.tile([128, 512], F32, tag="areps")
                    nc.tensor.matmul(aps, sel[:, b2 * 4 + m, :], alpb,
                                     start=True, stop=True,
                                     skip_group_check=True)
                    nc.vector.tensor_copy(
                        out=arep[:, m * 512:(m + 1) * 512], in_=aps)
                hacc = small.tile([128, HC], F32, tag="hacc")
                nc.vector.memset(hacc, 0.0)
                prod = prodp.tile([128, N], BF16, tag="prod")
                for c in range(HC):
                    nc.vector.scalar_tensor_tensor(
                        out=prod, in0=ck[:, c, :], scalar=1.0,
                        in1=arep,
                        op0=ALU.mult, op1=ALU.mult,
                        accum_out=hacc[:, c:c + 1])
                nc.vector.tensor_tensor(out=hacc, in0=hacc, in1=bc_p,
                                        op=ALU.add)
                for c in range(HC):
                    nc.sync.dma_start(
                        out=hid_e.ap()[b, c * 128:(c + 1) * 128],
                        in_=hacc[:, c:c + 1])

        pending = None

        # ---- phase 2: main loop ----
        for g in range(NGROUPS):
            attp = att_ps.tile([8, 512], F32, tag="attp")
            cks = []
            for b2 in range(2):
                b = 2 * g + b2
                ctxts = []
                for d in range(DC):
                    ct = stream.tile([128, N], BF16, tag="ctxt")
                    nc.sync.dma_start(out=ct,
                                      in_=ctxt_e.ap()[b, d * 128:(d + 1) * 128, :])
                    ctxts.append(ct)
                ck = keep.tile([128, HC, N], BF16, tag="ck")
                cks.append(ck)
                for npair in range(2):
                    for c in range(HC):
                        ps = ctx_ps.tile([128, 1024], F32, tag="ctxps")
                        for nh in range(2):
                            n0 = npair * 1024 + nh * 512
                            for d in range(DC):
                                nc.tensor.matmul(
                                    ps[:, nh * 512:(nh + 1) * 512],
                                    wct[:, d, c * 128:(c + 1) * 128],
                                    ctxts[d][:, n0:n0 + 512],
                                    start=(d == 0), stop=(d == DC - 1),
                                    skip_group_check=True)
                        t_t = tpool.tile([128, 1024], BF16, tag="t")
                        nc.scalar.activation(out=t_t, in_=ps, func=AF.Tanh,
                                             bias=inp_sb[:, c, b:b + 1], scale=1.0)
                        nc.scalar.activation(
                            out=ck[:, c, npair * 1024:(npair + 1) * 1024],
                            in_=ps, func=AF.Copy)
                        for nh in range(2):
                            m = npair * 2 + nh
                            slot = b2 * 4 + m
                            nc.tensor.matmul(
                                attp, voh[:, c, slot, :],
                                t_t[:, nh * 512:(nh + 1) * 512],
                                start=(b2 == 0 and npair == 0 and c == 0 and nh == 0),
                                stop=(b2 == 1 and npair == 1 and c == HC - 1 and nh == 1),
                                skip_group_check=True)
                if b2 == 0 and pending is not None:
                    emit_hidden(pending[0], pending[1], g - 1)
                    pending = None

            # ---- group softmax: rows r = b2*4 + n_tile ----
            mbg = mid.tile([8, 512], F32, tag="mbg")
            nc.sync.dma_start(out=mbg, in_=mb_e.ap()[:, g, :])
            attm = attm_all[:, g, :]
            nc.vector.tensor_tensor(out=attm, in0=attp, in1=mbg,
                                    op=ALU.add)
            nm8 = small.tile([8, 1], F32, tag="nm8")
            nc.vector.tensor_reduce(out=nm8, in_=attm, axis=AX.X, op=ALU.max,
                                    negate=True)
            e_t = mid.tile([8, 512], F32, tag="e")
            s8 = small.tile([8, 1], F32, tag="s8")
            nc.scalar.activation(out=e_t, in_=attm, func=AF.Exp, bias=nm8,
                                 scale=1.0, accum_out=s8)
            st8 = small.tile([8, 2], F32, tag="st8")
            nc.vector.tensor_copy(out=st8[:, 0:1], in_=nm8)
            nc.vector.tensor_copy(out=st8[:, 1:2], in_=s8)
            nc.sync.dma_start(out=scr_stats[g], in_=st8)
            # per-batch fixup on 2 partitions (b on partitions, n_tile on free)
            stf2 = small.tile([2, 4, 2], F32, tag="stf2")
            nc.sync.dma_start(out=stf2, in_=scr_stats[g])
            nmf2 = stf2[:, :, 0]
            sf2 = stf2[:, :, 1]
            nmb2 = stats_all[:, g, 0:1]
            nc.vector.tensor_reduce(out=nmb2, in_=nmf2, axis=AX.X, op=ALU.min)
            earg = small.tile([2, 4], F32, tag="earg")
            nc.vector.tensor_scalar_sub(out=earg, in0=nmf2, scalar1=nmb2)
            w2 = small.tile([2, 4], F32, tag="w2")
            nc.scalar.activation(out=w2, in_=earg, func=AF.Exp, scale=-1.0)
            sw2 = small.tile([2, 4], F32, tag="sw2")
            nc.vector.tensor_mul(out=sw2, in0=w2, in1=sf2)
            sb2 = stats_all[:, g, 1:2]
            nc.vector.tensor_reduce(out=sb2, in_=sw2, axis=AX.X, op=ALU.add)
            rb2 = small.tile([2, 1], F32, tag="rb2")
            nc.vector.reciprocal(out=rb2, in_=sb2)
            scale2 = small.tile([2, 4], F32, tag="scale2")
            nc.vector.tensor_scalar_mul(out=scale2, in0=w2, scalar1=rb2)
            nc.sync.dma_start(out=scr_scale[g], in_=scale2)
            sclp = small.tile([8, 1], F32, tag="sclp")
            nc.sync.dma_start(out=sclp, in_=scr_scale[g])
            alp = mid.tile([8, 512], F32, tag="alp")
            nc.vector.tensor_scalar_mul(out=alp, in0=e_t, scalar1=sclp)
            alpb = mid.tile([8, 512], BF16, tag="alpb")
            nc.vector.tensor_copy(out=alpb, in_=alp)
            for b2 in range(2):
                b = 2 * g + b2
                nc.sync.dma_start(
                    out=alp_e.ap()[b].rearrange("(m j) -> m j", m=NT),
                    in_=alp[b2 * 4:(b2 + 1) * 4, :])

            pending = (cks, alpb)

        # ---- tail: log_p = attm + (-M_b - ln s_b), one Ln for all groups ----
        lnall = const.tile([2, NGROUPS], F32)
        _ = None
        nc.scalar.activation(out=lnall, in_=stats_all[:, :, 1], func=AF.Ln)
        bias2all = const.tile([2, NGROUPS], F32)
        nc.vector.tensor_sub(out=bias2all, in0=stats_all[:, :, 0], in1=lnall)
        for g in range(NGROUPS):
            b2f = small.tile([2, 4], F32, tag="b2f")
            nc.vector.tensor_scalar_add(out=b2f, in0=zeros24,
                                        scalar1=bias2all[:, g:g + 1])
            nc.sync.dma_start(out=scr_b2g[g], in_=b2f)
            b2rep = small.tile([8, 1], F32, tag="b2rep")
            nc.sync.dma_start(out=b2rep, in_=scr_b2g[g])
            lgp = mid.tile([8, 512], F32, tag="lgp")
            nc.vector.tensor_scalar_add(out=lgp, in0=attm_all[:, g, :],
                                        scalar1=b2rep)
            for b2 in range(2):
                b = 2 * g + b2
                nc.sync.dma_start(
                    out=lgp_e.ap()[b].rearrange("(m j) -> m j", m=NT),
                    in_=lgp[b2 * 4:(b2 + 1) * 4, :])

        if pending is not None:
            emit_hidden(pending[0], pending[1], NGROUPS - 1)
            pending = None

    nc.compile()
    return nc


def _get_nc():
    if "nc" not in _CACHE:
        _CACHE["nc"] = _build_nc()
    return _CACHE["nc"]


def _prep_inputs(x, context, mask, Wi, bi, Wc, bc, V):
    """Host-side sharding + layout prep. Returns per-core input maps."""
    x = np.asarray(x, dtype=np.float32)
    context = np.asarray(context, dtype=np.float32)
    mask_b = np.asarray(mask).reshape(B, N)
    Wi = np.asarray(Wi, dtype=np.float32)
    bi = np.asarray(bi, dtype=np.float32)
    Wc = np.asarray(Wc, dtype=np.float32)
    bc = np.asarray(bc, dtype=np.float32)
    V = np.asarray(V, dtype=np.float32)

    # shared weights
    import ml_dtypes
    wct = np.ascontiguousarray(
        Wc.T.reshape(DC, 128, H).transpose(1, 0, 2)).astype(ml_dtypes.bfloat16)
    wit = np.ascontiguousarray(
        Wi.T.reshape(DC, 128, H).transpose(1, 0, 2)).astype(ml_dtypes.bfloat16)
    bi_p = np.ascontiguousarray(bi.reshape(HC, 128).T)         # [128, HC]
    bc_p = np.ascontiguousarray(bc.reshape(HC, 128).T)
    v_p = np.ascontiguousarray(V.reshape(HC, 128).T)

    mbias = np.where(mask_b, np.float32(-1e30), np.float32(0.0)).astype(np.float32)

    sel = np.zeros((8, 8, 128), dtype=ml_dtypes.bfloat16)
    for r in range(8):
        sel[r, r, :] = 1.0

    in_maps = []
    for i in range(NCORES):
        sl = slice(i * BPC, (i + 1) * BPC)
        ctxt = np.ascontiguousarray(
            context[sl].transpose(0, 2, 1)).astype(ml_dtypes.bfloat16)
        xt = np.ascontiguousarray(
            x[sl].T.reshape(DC, 128, BPC).transpose(1, 0, 2)).astype(ml_dtypes.bfloat16)
        mb_core = mbias[sl].reshape(NGROUPS, 2, NT, 512)
        mb_core = np.ascontiguousarray(
            mb_core.transpose(1, 2, 0, 3).reshape(8, NGROUPS, 512))
        in_maps.append({
            "ctxt": ctxt, "wct": wct, "wit": wit, "xt": xt,
            "bi_p": bi_p, "bc_p": bc_p, "v_p": v_p, "mb": mb_core,
            "sel": sel,
        })
    return in_maps, mask_b


def run(x, context, mask, Wi, bi, Wc, bc, V, trace=False, tmpdir=None):
    from concourse.bass_utils import run_bass_kernel_spmd

    nc = _get_nc()
    in_maps, mask_b = _prep_inputs(x, context, mask, Wi, bi, Wc, bc, V)
    kw = {}
    if trace:
        kw = dict(trace=True, tmpdir=tmpdir)
    res = run_bass_kernel_spmd(nc, in_maps, list(range(NCORES)), **kw)

    hidden = np.empty((B, H), dtype=np.float32)
    alpha = np.empty((B, N), dtype=np.float32)
    log_p = np.empty((B, N), dtype=np.float32)
    for i in range(NCORES):
        sl = slice(i * BPC, (i + 1) * BPC)
        r = res.results[i]
        hidden[sl] = r["hidden"]
        alpha[sl] = r["alpha"]
        log_p[sl] = r["logp"]
    alpha = np.where(mask_b, np.float32(0.0), alpha)
    log_p = np.where(mask_b, np.float32(-np.inf), log_p)
    return (hidden, alpha, log_p), res


def kernel(x, context, mask, Wi, bi, Wc, bc, V):
    out, _ = run(x, context, mask, Wi, bi, Wc, bc, V, trace=False)
    return out
